# revision 9
# baseline (speedup 1.0000x reference)
"""Distributed Trainium2 kernel for the attention block (8 NeuronCores).

Sharding: core i owns KV head i and Q heads [4i, 4i+4) (tensor parallel over
heads). Attention outputs are transposed on the fly and exchanged with an
AllToAll so each core computes the o_proj for its 256-row slice of the
sequence; host concatenates slices.

All matmuls run in bf16 with fp32 PSUM accumulation; softmax statistics and
norms in fp32. RoPE + QK RMS-norm are folded into precomputed tables and
per-row scales.
"""
from contextlib import ExitStack

import numpy as np
import ml_dtypes

import concourse.bass as bass
import concourse.bacc as bacc
import concourse.tile as tile
from concourse.tile import add_dep_helper
from concourse import mybir
from concourse.bass_utils import run_bass_kernel_spmd

BF16 = mybir.dt.bfloat16
F32 = mybir.dt.float32

N_CORES = 8
S = 2048
D_MODEL = 2560
HEAD_DIM = 128
NUM_HEADS = 32
NUM_KV_HEADS = 8
Q_PER_CORE = NUM_HEADS // N_CORES          # 4
QKV_R = Q_PER_CORE * HEAD_DIM + 2 * HEAD_DIM  # 768 rows per core (4q + k + v)
QK_R = Q_PER_CORE * HEAD_DIM + HEAD_DIM       # 640 (q + k, normed+roped)
ROPE_THETA = 5e6
EPS = 1e-6
N_ST = S // 128          # 16 seq tiles
N_DC = D_MODEL // 128    # 20 contraction chunks
S_SLICE = S // N_CORES   # 256 output rows per core
N_HD = (NUM_HEADS * HEAD_DIM) // 128  # 32 o_proj contraction chunks
AV_W = 132               # 128 v cols + 1 ones + 3 pad
E_HALF = D_MODEL // 2    # 1280

_CACHE = {}


def _build():
    nc = bacc.Bacc("TRN2", target_bir_lowering=False, debug=False,
                   num_devices=N_CORES)

    xT = nc.dram_tensor("xT", [D_MODEL, S], BF16, kind="ExternalInput")
    wqkvT = nc.dram_tensor("wqkvT", [D_MODEL, QKV_R], BF16, kind="ExternalInput")
    woT = nc.dram_tensor("woT", [NUM_HEADS * HEAD_DIM, D_MODEL], BF16,
                         kind="ExternalInput")
    cosT = nc.dram_tensor("cosT", [S, QK_R], BF16, kind="ExternalInput")
    sinT = nc.dram_tensor("sinT", [S, QK_R], BF16, kind="ExternalInput")
    tri = nc.dram_tensor("tri", [128, 128], F32, kind="ExternalInput")

    out_slice = nc.dram_tensor("out_slice", [S_SLICE, D_MODEL], F32,
                               kind="ExternalOutput")
    k_cache = nc.dram_tensor("k_cache", [S, HEAD_DIM], F32, kind="ExternalOutput")
    v_cache = nc.dram_tensor("v_cache", [S, HEAD_DIM], F32, kind="ExternalOutput")

    with tile.TileContext(nc) as tc:
        with (
            tc.tile_pool(name="persist", bufs=1) as pp,
            tc.tile_pool(name="vaug", bufs=N_ST) as vaug_pool,
            tc.tile_pool(name="dram", bufs=1, space="DRAM") as dram,
        ):
            # persistent tiles
            qkT_all = pp.tile([128, 5, S], BF16)    # [d, block(4q+k), s]
            v_sb = [vaug_pool.tile([128, HEAD_DIM], BF16, tag="vaug", name=f"vsb{t}")
                     for t in range(N_ST)]
            tri_sb = pp.tile([128, 128], F32)
            nc.sync.dma_start(out=tri_sb[:], in_=tri[:])
            eps_sb = pp.tile([128, 1], F32)
            nc.vector.memset(eps_sb, float(HEAD_DIM) * EPS)


            a2a_in = dram.tile([N_CORES, Q_PER_CORE * HEAD_DIM, S_SLICE], BF16)
            a2a_out = dram.tile([N_CORES, Q_PER_CORE * HEAD_DIM, S_SLICE], BF16)

            # ---------------- Phase A: QKV projection + norm + rope ----------
            with (
                tc.tile_pool(name="xt", bufs=N_DC) as xt_pool,
                tc.tile_pool(name="wk", bufs=N_DC) as wk_pool,
                tc.tile_pool(name="tabs", bufs=3) as tab_pool,
                tc.tile_pool(name="qkwork", bufs=4) as work_pool,
                tc.tile_pool(name="stats", bufs=6) as stat_pool,
                tc.tile_pool(name="apsum", bufs=2, space="PSUM") as apsum,
            ):
                xT_sb = [xt_pool.tile([128, S], BF16, tag="xt", name=f"xTs{d}")
                         for d in range(N_DC)]
                wq_sb = [wk_pool.tile([128, QKV_R], BF16, tag="wk", name=f"wqs{d}")
                         for d in range(N_DC)]
                for d in range(N_DC):
                    nc.sync.dma_start(out=xT_sb[d][:], in_=xT[128 * d:128 * (d + 1), :])
                    nc.sync.dma_start(out=wq_sb[d][:], in_=wqkvT[128 * d:128 * (d + 1), :])

                for t in range(N_ST):
                    ps = apsum.tile([128, QKV_R], F32, tag="qkvps")
                    for d in range(N_DC):
                        lhsT = xT_sb[d][:, 128 * t:128 * (t + 1)]
                        nc.tensor.matmul(ps[:, 0:512], lhsT, wq_sb[d][:, 0:512],
                                         start=(d == 0), stop=(d == N_DC - 1))
                        nc.tensor.matmul(ps[:, 512:QKV_R], lhsT, wq_sb[d][:, 512:QKV_R],
                                         start=(d == 0), stop=(d == N_DC - 1))

                    # v: fp32 cache out + bf16 for AV
                    v32 = work_pool.tile([128, HEAD_DIM], F32, tag="v32")
                    nc.vector.tensor_copy(v32[:], ps[:, 640:768])
                    nc.gpsimd.dma_start(out=v_cache[128 * t:128 * (t + 1), :],
                                        in_=v32[:])
                    nc.vector.tensor_copy(v_sb[t][:], v32[:])

                    # rms-norm stats for 4 q heads + k
                    sums = stat_pool.tile([128, 5], F32, tag="sums")
                    scr = work_pool.tile([128, 128], BF16, tag="sqscr")
                    for b in range(5):
                        nc.scalar.activation(
                            out=scr[:], in_=ps[:, 128 * b:128 * (b + 1)],
                            func=mybir.ActivationFunctionType.Square,
                            accum_out=sums[:, b:b + 1])
                    inv = stat_pool.tile([128, 5], F32, tag="inv")
                    nc.scalar.activation(out=inv[:], in_=sums[:],
                                         func=mybir.ActivationFunctionType.Sqrt,
                                         bias=eps_sb[:])
                    nc.vector.reciprocal(inv[:], inv[:])
                    # k scale needs * sqrt(HEAD_DIM) (q keeps 1/sqrt(d) fold)
                    nc.vector.tensor_scalar_mul(inv[:, 4:5], inv[:, 4:5],
                                                float(np.sqrt(HEAD_DIM)))

                    # evacuate q,k with norm scale applied -> bf16
                    qk_n = work_pool.tile([128, QK_R], BF16, tag="qkn")
                    for b in range(5):
                        nc.scalar.activation(
                            out=qk_n[:, 128 * b:128 * (b + 1)],
                            in_=ps[:, 128 * b:128 * (b + 1)],
                            func=mybir.ActivationFunctionType.Copy,
                            scale=inv[:, b:b + 1])

                    # rope
                    cos_sb = tab_pool.tile([128, QK_R], BF16, tag="cos")
                    sin_sb = tab_pool.tile([128, QK_R], BF16, tag="sin")
                    nc.sync.dma_start(out=cos_sb[:], in_=cosT[128 * t:128 * (t + 1), :])
                    nc.sync.dma_start(out=sin_sb[:], in_=sinT[128 * t:128 * (t + 1), :])
                    qk_r = work_pool.tile([128, QK_R], BF16, tag="qkr")
                    tmp = work_pool.tile([128, QK_R], BF16, tag="ropetmp")
                    qn3 = qk_n.rearrange("p (b h) -> p b h", b=5)
                    qr3 = qk_r.rearrange("p (b h) -> p b h", b=5)
                    sn3 = sin_sb.rearrange("p (b h) -> p b h", b=5)
                    nc.vector.tensor_mul(qr3[:, :, 0:64], qn3[:, :, 64:128],
                                         sn3[:, :, 0:64])
                    nc.vector.tensor_mul(qr3[:, :, 64:128], qn3[:, :, 0:64],
                                         sn3[:, :, 64:128])
                    nc.vector.tensor_mul(tmp[:], qk_n[:], cos_sb[:])
                    nc.vector.tensor_add(qk_r[:], qk_r[:], tmp[:])

                    # k cache (fp32)
                    k32 = work_pool.tile([128, HEAD_DIM], F32, tag="k32")
                    nc.vector.tensor_copy(k32[:], qk_r[:, 512:640])
                    nc.gpsimd.dma_start(out=k_cache[128 * t:128 * (t + 1), :],
                                        in_=k32[:])

                    # batched transpose: all 5 blocks in one xbar call
                    nc.sync.dma_start_transpose(
                        out=qkT_all[:, :, 128 * t:128 * (t + 1)], in_=qk_r[:])

            # ---------------- Phase B: attention (natural softmax, v-stationary AV)
            wo_stack = ExitStack()
            wo_pool = wo_stack.enter_context(tc.tile_pool(name="wo", bufs=12))
            with (
                tc.tile_pool(name="ptt", bufs=1) as ptt_pool,
                tc.tile_pool(name="pt", bufs=3) as pt_pool,
                tc.tile_pool(name="asb", bufs=2) as a_pool,
                tc.tile_pool(name="den", bufs=8) as den_pool,
                tc.tile_pool(name="spsum", bufs=2, space="PSUM") as spsum,
                tc.tile_pool(name="avpsum", bufs=1, space="PSUM") as avpsum,
            ):
                # P^T storage: c-slot layout, slot c holds blocks (c, t>=c) at
                # col c*S + (t-c)*128  -> strip-t transpose writes with uniform
                # j-stride of (S - 128) elements; AV reads slot c contiguously.
                ptt = ptt_pool.tile([128, N_ST * S], BF16, name="ptt")
                ptt3 = ptt.rearrange("p (c w) -> p c w", c=N_ST)
                prev_evac = None
                for h in range(Q_PER_CORE):
                    tr_insts = []
                    for t in range(N_ST):
                        width = 128 * (t + 1)
                        pt = pt_pool.tile([128, 2048], BF16, tag="pt",
                                          name=f"pt{h}_{t}")
                        denp = den_pool.tile([128, 2], F32, tag="denp",
                                             name=f"denp{h}_{t}")
                        nch = (width + 1023) // 1024
                        for jc in range(nch):
                            j0 = 1024 * jc
                            w = min(1024, width - j0)
                            sps = spsum.tile([128, 1024], F32, tag="sps")
                            for e in range(0, w, 512):
                                we = min(512, w - e)
                                nc.tensor.matmul(
                                    sps[:, e:e + we],
                                    qkT_all[:, h, 128 * t:128 * (t + 1)],
                                    qkT_all[:, 4, j0 + e:j0 + e + we],
                                    start=True, stop=True)
                            if jc == nch - 1:
                                # additive causal mask on the diagonal block
                                nc.vector.tensor_add(sps[:, w - 128:w],
                                                     sps[:, w - 128:w], tri_sb[:])
                            nc.scalar.activation(
                                out=pt[:, j0:j0 + w], in_=sps[:, 0:w],
                                func=mybir.ActivationFunctionType.Exp,
                                accum_out=denp[:, jc:jc + 1])
                        den = den_pool.tile([128, 1], F32, tag="den",
                                            name=f"den{h}_{t}")
                        if nch == 2:
                            nc.vector.tensor_add(den[:], denp[:, 0:1], denp[:, 1:2])
                            nc.vector.reciprocal(den[:], den[:])
                        else:
                            nc.vector.reciprocal(den[:], denp[:, 0:1])
                        nc.vector.tensor_scalar_mul(pt[:, 0:width], pt[:, 0:width],
                                                    den[:])
                        # one xbar call: strip t -> P^T blocks (c=0..t, t)
                        eng = nc.sync if t % 2 == 0 else nc.scalar
                        base = ptt[:]
                        dst = bass.AP(
                            tensor=base.tensor, offset=base.offset + 128 * t,
                            ap=[[base.ap[0][0], 128], [S - 128, t + 1], [1, 128]])
                        tr = eng.dma_start_transpose(out=dst, in_=pt[:, 0:width])
                        if prev_evac is not None:
                            add_dep_helper(tr.ins, prev_evac.ins,
                                           reason="ptt WAR across heads")
                        tr_insts.append(tr)

                    # AV: v_c stationary, wide P^T streams -> out^T accumulate
                    avT = avpsum.tile([128, S], F32, tag="avt", name=f"avt{h}")
                    for c in range(N_ST):
                        sq = 128 * c
                        while sq < S:
                            w = min(512 - (sq % 512), S - sq)
                            last_c = min(N_ST - 1, 4 * (sq // 512) + 3)
                            mm = nc.tensor.matmul(
                                avT[:, sq:sq + w], v_sb[c][:],
                                ptt3[:, c, sq - 128 * c:sq - 128 * c + w],
                                start=(c == 0), stop=(c == last_c))
                            for ti in tr_insts:
                                add_dep_helper(mm.ins, ti.ins,
                                               reason="AV reads ptt transposes")
                            sq += w
                    at_h = a_pool.tile([128, S], BF16, tag="ath", name=f"ath{h}")
                    prev_evac = nc.vector.tensor_copy(at_h[:], avT[:])
                    for j in range(N_CORES):
                        nc.gpsimd.dma_start(
                            out=a2a_in[j, 128 * h:128 * (h + 1), :],
                            in_=at_h[:, S_SLICE * j:S_SLICE * (j + 1)])

                nc.gpsimd.collective_compute(
                    "AllToAll", mybir.AluOpType.bypass,
                    replica_groups=[list(range(N_CORES))],
                    ins=[a2a_in.opt()], outs=[a2a_out.opt()])

            # ---------------- Phase C: o_proj for our seq slice -------------
            with (
                tc.tile_pool(name="at", bufs=N_HD) as at_pool,
                tc.tile_pool(name="osb", bufs=4) as o_pool,
                tc.tile_pool(name="opsum", bufs=2, space="PSUM") as opsum,
            ):
                at_tiles = []
                for m in range(N_HD):
                    j, mm = divmod(m, Q_PER_CORE)
                    at = at_pool.tile([128, S_SLICE], BF16, tag="at",
                                      name=f"at{m}")
                    at_tiles.append(at)
                    nc.sync.dma_start(
                        out=at[:], in_=a2a_out[j, 128 * mm:128 * (mm + 1), :])

                for eh in range(2):
                    e0 = eh * E_HALF
                    psos = [opsum.tile([128, E_HALF], F32, tag="pso",
                                       name=f"pso{eh}_{sc}") for sc in range(2)]
                    for m in range(N_HD):
                        wo_sb = wo_pool.tile([128, E_HALF], BF16, tag="wo",
                                             name=f"wo{eh}_{m}")
                        nc.sync.dma_start(out=wo_sb[:],
                                          in_=woT[128 * m:128 * (m + 1),
                                                  e0:e0 + E_HALF])
                        for sc in range(2):
                            for e in range(0, E_HALF, 512):
                                w = min(512, E_HALF - e)
                                nc.tensor.matmul(
                                    psos[sc][:, e:e + w],
                                    at_tiles[m][:, 128 * sc:128 * (sc + 1)],
                                    wo_sb[:, e:e + w],
                                    start=(m == 0), stop=(m == N_HD - 1))
                    for sc in range(2):
                        for e in range(0, E_HALF, 640):
                            ob = o_pool.tile([128, 640], F32, tag="osb")
                            nc.vector.tensor_copy(ob[:], psos[sc][:, e:e + 640])
                            nc.sync.dma_start(
                                out=out_slice[128 * sc:128 * (sc + 1),
                                              e0 + e:e0 + e + 640],
                                in_=ob[:])
            wo_stack.close()

    nc.compile()
    return nc


def _host_prep(x, Wq, Wk, Wv, Wo, q_norm_w, k_norm_w):
    bf = ml_dtypes.bfloat16
    x2 = np.asarray(x, np.float32).reshape(S, D_MODEL)
    xT = np.ascontiguousarray(x2.T).astype(bf)
    woT = np.ascontiguousarray(np.asarray(Wo, np.float32).T).astype(bf)

    # rope tables with norm weights + rotate-half sign folded in
    pos = np.arange(S, dtype=np.float64)
    inv_freq = 1.0 / (ROPE_THETA ** (np.arange(0, HEAD_DIM, 2, dtype=np.float64)
                                     / HEAD_DIM))
    ang = pos[:, None] * inv_freq[None, :]          # (S, 64)
    cos = np.concatenate([np.cos(ang), np.cos(ang)], axis=1)  # (S, 128)
    sin = np.concatenate([np.sin(ang), np.sin(ang)], axis=1)
    sgn = np.where(np.arange(HEAD_DIM) < 64, -1.0, 1.0)[None, :]
    rot_idx = (np.arange(HEAD_DIM) + 64) % HEAD_DIM
    qw = np.asarray(q_norm_w, np.float64)
    kw = np.asarray(k_norm_w, np.float64)
    cos_q = cos * qw[None, :]
    sin_q = sin * sgn * qw[rot_idx][None, :]
    cos_k = cos * kw[None, :]
    sin_k = sin * sgn * kw[rot_idx][None, :]
    cosT = np.concatenate([np.tile(cos_q, (1, Q_PER_CORE)), cos_k], axis=1).astype(bf)
    sinT = np.concatenate([np.tile(sin_q, (1, Q_PER_CORE)), sin_k], axis=1).astype(bf)

    tri = (np.triu(np.ones((128, 128), np.float32), k=1) * -60.0)  # -60 where sk > sq

    Wq = np.asarray(Wq, np.float32)
    Wk = np.asarray(Wk, np.float32)
    Wv = np.asarray(Wv, np.float32)
    in_maps = []
    for i in range(N_CORES):
        w_i = np.concatenate([
            Wq[Q_PER_CORE * HEAD_DIM * i: Q_PER_CORE * HEAD_DIM * (i + 1)],
            Wk[HEAD_DIM * i: HEAD_DIM * (i + 1)],
            Wv[HEAD_DIM * i: HEAD_DIM * (i + 1)],
        ], axis=0)                                   # (768, 2560)
        wqkvT = np.ascontiguousarray(w_i.T).astype(bf)
        in_maps.append({
            "xT": xT, "wqkvT": wqkvT, "woT": woT,
            "cosT": cosT, "sinT": sinT, "tri": tri,
        })
    return in_maps


def kernel(x, Wq, Wk, Wv, Wo, q_norm_w, k_norm_w, _trace=False, _trace_out=None):
    if "nc" not in _CACHE:
        _CACHE["nc"] = _build()
    nc = _CACHE["nc"]
    in_maps = _host_prep(x, Wq, Wk, Wv, Wo, q_norm_w, k_norm_w)
    kw = {}
    if _trace:
        kw = dict(trace=True)
        if _trace_out:
            kw["tmpdir"] = _trace_out
    res = run_bass_kernel_spmd(nc, in_maps, list(range(N_CORES)), **kw)
    _CACHE["last_exec_ns"] = res.exec_time_ns
    r = res.results
    out = np.concatenate([r[i]["out_slice"] for i in range(N_CORES)], axis=0)
    out = out.reshape(1, S, D_MODEL)
    kc = np.stack([r[i]["k_cache"] for i in range(N_CORES)], axis=0)[None]
    vc = np.stack([r[i]["v_cache"] for i in range(N_CORES)], axis=0)[None]
    return (out, kc, vc)


# revision 10
# speedup vs baseline: 1.0195x; 1.0195x over previous
"""Distributed Trainium2 kernel for the attention block (8 NeuronCores).

Sharding: core i owns KV head i and Q heads [4i, 4i+4) (tensor parallel over
heads). Attention outputs are transposed on the fly and exchanged with an
AllToAll so each core computes the o_proj for its 256-row slice of the
sequence; host concatenates slices.

All matmuls run in bf16 with fp32 PSUM accumulation; softmax statistics and
norms in fp32. RoPE + QK RMS-norm are folded into precomputed tables and
per-row scales.
"""
from contextlib import ExitStack

import numpy as np
import ml_dtypes

import concourse.bass as bass
import concourse.bacc as bacc
import concourse.tile as tile
from concourse.tile import add_dep_helper
from concourse import mybir
from concourse.bass_utils import run_bass_kernel_spmd

BF16 = mybir.dt.bfloat16
F32 = mybir.dt.float32

N_CORES = 8
S = 2048
D_MODEL = 2560
HEAD_DIM = 128
NUM_HEADS = 32
NUM_KV_HEADS = 8
Q_PER_CORE = NUM_HEADS // N_CORES          # 4
QKV_R = Q_PER_CORE * HEAD_DIM + 2 * HEAD_DIM  # 768 rows per core (4q + k + v)
QK_R = Q_PER_CORE * HEAD_DIM + HEAD_DIM       # 640 (q + k, normed+roped)
ROPE_THETA = 5e6
EPS = 1e-6
N_ST = S // 128          # 16 seq tiles
N_DC = D_MODEL // 128    # 20 contraction chunks
S_SLICE = S // N_CORES   # 256 output rows per core
N_HD = (NUM_HEADS * HEAD_DIM) // 128  # 32 o_proj contraction chunks
AV_W = 132               # 128 v cols + 1 ones + 3 pad
E_HALF = D_MODEL // 2    # 1280

_CACHE = {}


def _build():
    nc = bacc.Bacc("TRN2", target_bir_lowering=False, debug=False,
                   num_devices=N_CORES)

    xT = nc.dram_tensor("xT", [D_MODEL, S], BF16, kind="ExternalInput")
    wqkvT = nc.dram_tensor("wqkvT", [D_MODEL, QKV_R], BF16, kind="ExternalInput")
    woT = nc.dram_tensor("woT", [NUM_HEADS * HEAD_DIM, D_MODEL], BF16,
                         kind="ExternalInput")
    cosT = nc.dram_tensor("cosT", [S, QK_R], BF16, kind="ExternalInput")
    sinT = nc.dram_tensor("sinT", [S, QK_R], BF16, kind="ExternalInput")
    tri = nc.dram_tensor("tri", [128, 128], F32, kind="ExternalInput")

    out_slice = nc.dram_tensor("out_slice", [S_SLICE, D_MODEL], F32,
                               kind="ExternalOutput")
    k_cache = nc.dram_tensor("k_cache", [S, HEAD_DIM], F32, kind="ExternalOutput")
    v_cache = nc.dram_tensor("v_cache", [S, HEAD_DIM], F32, kind="ExternalOutput")

    with tile.TileContext(nc) as tc:
        with (
            tc.tile_pool(name="persist", bufs=1) as pp,
            tc.tile_pool(name="vaug", bufs=N_ST) as vaug_pool,
            tc.tile_pool(name="dram", bufs=1, space="DRAM") as dram,
        ):
            # persistent tiles
            qkT_all = pp.tile([128, 5, S], BF16)    # [d, block(4q+k), s]
            v_sb = [vaug_pool.tile([128, HEAD_DIM], BF16, tag="vaug", name=f"vsb{t}")
                     for t in range(N_ST)]
            tri_sb = pp.tile([128, 128], F32)
            nc.sync.dma_start(out=tri_sb[:], in_=tri[:])
            eps_sb = pp.tile([128, 1], F32)
            nc.vector.memset(eps_sb, float(HEAD_DIM) * EPS)


            a2a_in = dram.tile([N_CORES, Q_PER_CORE * HEAD_DIM, S_SLICE], BF16)
            a2a_out = dram.tile([N_CORES, Q_PER_CORE * HEAD_DIM, S_SLICE], BF16)

            # ---------------- Phase A: QKV projection + norm + rope ----------
            with (
                tc.tile_pool(name="xt", bufs=N_DC) as xt_pool,
                tc.tile_pool(name="wk", bufs=N_DC) as wk_pool,
                tc.tile_pool(name="tabs", bufs=3) as tab_pool,
                tc.tile_pool(name="qkwork", bufs=4) as work_pool,
                tc.tile_pool(name="stats", bufs=6) as stat_pool,
                tc.tile_pool(name="apsum", bufs=2, space="PSUM") as apsum,
            ):
                xT_sb = [xt_pool.tile([128, S], BF16, tag="xt", name=f"xTs{d}")
                         for d in range(N_DC)]
                wq_sb = [wk_pool.tile([128, QKV_R], BF16, tag="wk", name=f"wqs{d}")
                         for d in range(N_DC)]
                for d in range(N_DC):
                    nc.sync.dma_start(out=xT_sb[d][:], in_=xT[128 * d:128 * (d + 1), :])
                    nc.sync.dma_start(out=wq_sb[d][:], in_=wqkvT[128 * d:128 * (d + 1), :])

                for t in range(N_ST):
                    ps = apsum.tile([128, QKV_R], F32, tag="qkvps")
                    for d in range(N_DC):
                        lhsT = xT_sb[d][:, 128 * t:128 * (t + 1)]
                        nc.tensor.matmul(ps[:, 0:512], lhsT, wq_sb[d][:, 0:512],
                                         start=(d == 0), stop=(d == N_DC - 1))
                        nc.tensor.matmul(ps[:, 512:QKV_R], lhsT, wq_sb[d][:, 512:QKV_R],
                                         start=(d == 0), stop=(d == N_DC - 1))

                    # v: fp32 cache out + bf16 for AV
                    v32 = work_pool.tile([128, HEAD_DIM], F32, tag="v32")
                    nc.vector.tensor_copy(v32[:], ps[:, 640:768])
                    nc.gpsimd.dma_start(out=v_cache[128 * t:128 * (t + 1), :],
                                        in_=v32[:])
                    nc.vector.tensor_copy(v_sb[t][:], v32[:])

                    # rms-norm stats for 4 q heads + k
                    sums = stat_pool.tile([128, 5], F32, tag="sums")
                    scr = work_pool.tile([128, 128], BF16, tag="sqscr")
                    for b in range(5):
                        nc.scalar.activation(
                            out=scr[:], in_=ps[:, 128 * b:128 * (b + 1)],
                            func=mybir.ActivationFunctionType.Square,
                            accum_out=sums[:, b:b + 1])
                    inv = stat_pool.tile([128, 5], F32, tag="inv")
                    nc.scalar.activation(out=inv[:], in_=sums[:],
                                         func=mybir.ActivationFunctionType.Sqrt,
                                         bias=eps_sb[:])
                    nc.vector.reciprocal(inv[:], inv[:])
                    # k scale needs * sqrt(HEAD_DIM) (q keeps 1/sqrt(d) fold)
                    nc.vector.tensor_scalar_mul(inv[:, 4:5], inv[:, 4:5],
                                                float(np.sqrt(HEAD_DIM)))

                    # evacuate q,k with norm scale applied -> bf16
                    qk_n = work_pool.tile([128, QK_R], BF16, tag="qkn")
                    for b in range(5):
                        nc.scalar.activation(
                            out=qk_n[:, 128 * b:128 * (b + 1)],
                            in_=ps[:, 128 * b:128 * (b + 1)],
                            func=mybir.ActivationFunctionType.Copy,
                            scale=inv[:, b:b + 1])

                    # rope
                    cos_sb = tab_pool.tile([128, QK_R], BF16, tag="cos")
                    sin_sb = tab_pool.tile([128, QK_R], BF16, tag="sin")
                    nc.sync.dma_start(out=cos_sb[:], in_=cosT[128 * t:128 * (t + 1), :])
                    nc.sync.dma_start(out=sin_sb[:], in_=sinT[128 * t:128 * (t + 1), :])
                    qk_r = work_pool.tile([128, QK_R], BF16, tag="qkr")
                    tmp = work_pool.tile([128, QK_R], BF16, tag="ropetmp")
                    qn3 = qk_n.rearrange("p (b h) -> p b h", b=5)
                    qr3 = qk_r.rearrange("p (b h) -> p b h", b=5)
                    sn3 = sin_sb.rearrange("p (b h) -> p b h", b=5)
                    nc.vector.tensor_mul(qr3[:, :, 0:64], qn3[:, :, 64:128],
                                         sn3[:, :, 0:64])
                    nc.vector.tensor_mul(qr3[:, :, 64:128], qn3[:, :, 0:64],
                                         sn3[:, :, 64:128])
                    nc.vector.tensor_mul(tmp[:], qk_n[:], cos_sb[:])
                    nc.vector.tensor_add(qk_r[:], qk_r[:], tmp[:])

                    # k cache (fp32)
                    k32 = work_pool.tile([128, HEAD_DIM], F32, tag="k32")
                    nc.vector.tensor_copy(k32[:], qk_r[:, 512:640])
                    nc.gpsimd.dma_start(out=k_cache[128 * t:128 * (t + 1), :],
                                        in_=k32[:])

                    # batched transpose: all 5 blocks in one xbar call
                    nc.sync.dma_start_transpose(
                        out=qkT_all[:, :, 128 * t:128 * (t + 1)], in_=qk_r[:])

            # ---------------- Phase B: attention (natural softmax, v-stationary AV)
            wo_stack = ExitStack()
            wo_pool = wo_stack.enter_context(tc.tile_pool(name="wo", bufs=20))
            with (
                tc.tile_pool(name="ptt", bufs=1) as ptt_pool,
                tc.tile_pool(name="pt", bufs=4) as pt_pool,
                tc.tile_pool(name="asb", bufs=2) as a_pool,
                tc.tile_pool(name="den", bufs=8) as den_pool,
                tc.tile_pool(name="spsum", bufs=2, space="PSUM") as spsum,
                tc.tile_pool(name="avpsum", bufs=1, space="PSUM") as avpsum,
            ):
                # P^T storage: c-slot layout, slot c holds blocks (c, t>=c) at
                # col c*S + (t-c)*128  -> strip-t transpose writes with uniform
                # j-stride of (S - 128) elements; AV reads slot c contiguously.
                ptt = ptt_pool.tile([128, N_ST * S], BF16, name="ptt")
                ptt3 = ptt.rearrange("p (c w) -> p c w", c=N_ST)
                prev_av_last = None
                for h in range(Q_PER_CORE):
                    tr_by_t = []
                    for t in range(N_ST):
                        width = 128 * (t + 1)
                        pt = pt_pool.tile([128, 2048], BF16, tag="pt",
                                          name=f"pt{h}_{t}")
                        denp = den_pool.tile([128, 2], F32, tag="denp",
                                             name=f"denp{h}_{t}")
                        nch = (width + 1023) // 1024
                        for jc in range(nch):
                            j0 = 1024 * jc
                            w = min(1024, width - j0)
                            sps = spsum.tile([128, 1024], F32, tag="sps")
                            for e in range(0, w, 512):
                                we = min(512, w - e)
                                nc.tensor.matmul(
                                    sps[:, e:e + we],
                                    qkT_all[:, h, 128 * t:128 * (t + 1)],
                                    qkT_all[:, 4, j0 + e:j0 + e + we],
                                    start=True, stop=True)
                            if jc == nch - 1:
                                # additive causal mask on the diagonal block
                                nc.vector.tensor_add(sps[:, w - 128:w],
                                                     sps[:, w - 128:w], tri_sb[:])
                            nc.scalar.activation(
                                out=pt[:, j0:j0 + w], in_=sps[:, 0:w],
                                func=mybir.ActivationFunctionType.Exp,
                                accum_out=denp[:, jc:jc + 1])
                        den = den_pool.tile([128, 1], F32, tag="den",
                                            name=f"den{h}_{t}")
                        if nch == 2:
                            nc.vector.tensor_add(den[:], denp[:, 0:1], denp[:, 1:2])
                            nc.vector.reciprocal(den[:], den[:])
                        else:
                            nc.vector.reciprocal(den[:], denp[:, 0:1])
                        nc.vector.tensor_scalar_mul(pt[:, 0:width], pt[:, 0:width],
                                                    den[:])
                        # one xbar call: strip t -> P^T blocks (c=0..t, t)
                        eng = nc.sync if t % 2 == 0 else nc.scalar
                        base = ptt[:]
                        dst = bass.AP(
                            tensor=base.tensor, offset=base.offset + 128 * t,
                            ap=[[base.ap[0][0], 128], [S - 128, t + 1], [1, 128]])
                        tr = eng.dma_start_transpose(out=dst, in_=pt[:, 0:width])
                        if prev_av_last is not None:
                            for c in range(t + 1):
                                add_dep_helper(tr.ins, prev_av_last[c].ins,
                                               reason="ptt WAR across heads")
                        tr_by_t.append(tr)

                    # AV: v_c stationary, wide P^T streams -> out^T accumulate
                    avT = avpsum.tile([128, S], F32, tag="avt", name=f"avt{h}")
                    av_last = [None] * N_ST
                    for c in range(N_ST):
                        sq = 128 * c
                        while sq < S:
                            w = min(512 - (sq % 512), S - sq)
                            last_c = min(N_ST - 1, 4 * (sq // 512) + 3)
                            mm = nc.tensor.matmul(
                                avT[:, sq:sq + w], v_sb[c][:],
                                ptt3[:, c, sq - 128 * c:sq - 128 * c + w],
                                start=(c == 0), stop=(c == last_c))
                            for t2 in range(sq // 128, (sq + w - 1) // 128 + 1):
                                add_dep_helper(mm.ins, tr_by_t[t2].ins,
                                               reason="AV reads ptt transposes")
                            av_last[c] = mm
                            sq += w
                    prev_av_last = av_last
                    at_h = a_pool.tile([128, S], BF16, tag="ath", name=f"ath{h}")
                    nc.vector.tensor_copy(at_h[:], avT[:])
                    for j in range(N_CORES):
                        nc.gpsimd.dma_start(
                            out=a2a_in[j, 128 * h:128 * (h + 1), :],
                            in_=at_h[:, S_SLICE * j:S_SLICE * (j + 1)])

                nc.gpsimd.collective_compute(
                    "AllToAll", mybir.AluOpType.bypass,
                    replica_groups=[list(range(N_CORES))],
                    ins=[a2a_in.opt()], outs=[a2a_out.opt()])

            # ---------------- Phase C: o_proj for our seq slice -------------
            with (
                tc.tile_pool(name="at", bufs=N_HD) as at_pool,
                tc.tile_pool(name="osb", bufs=4) as o_pool,
                tc.tile_pool(name="opsum", bufs=2, space="PSUM") as opsum,
            ):
                at_tiles = []
                for m in range(N_HD):
                    j, mm = divmod(m, Q_PER_CORE)
                    at = at_pool.tile([128, S_SLICE], BF16, tag="at",
                                      name=f"at{m}")
                    at_tiles.append(at)
                    nc.sync.dma_start(
                        out=at[:], in_=a2a_out[j, 128 * mm:128 * (mm + 1), :])

                for eh in range(2):
                    e0 = eh * E_HALF
                    psos = [opsum.tile([128, E_HALF], F32, tag="pso",
                                       name=f"pso{eh}_{sc}") for sc in range(2)]
                    for m in range(N_HD):
                        wo_sb = wo_pool.tile([128, E_HALF], BF16, tag="wo",
                                             name=f"wo{eh}_{m}")
                        nc.sync.dma_start(out=wo_sb[:],
                                          in_=woT[128 * m:128 * (m + 1),
                                                  e0:e0 + E_HALF])
                        for sc in range(2):
                            for e in range(0, E_HALF, 512):
                                w = min(512, E_HALF - e)
                                nc.tensor.matmul(
                                    psos[sc][:, e:e + w],
                                    at_tiles[m][:, 128 * sc:128 * (sc + 1)],
                                    wo_sb[:, e:e + w],
                                    start=(m == 0), stop=(m == N_HD - 1))
                    for sc in range(2):
                        for e in range(0, E_HALF, 640):
                            ob = o_pool.tile([128, 640], F32, tag="osb")
                            nc.vector.tensor_copy(ob[:], psos[sc][:, e:e + 640])
                            nc.sync.dma_start(
                                out=out_slice[128 * sc:128 * (sc + 1),
                                              e0 + e:e0 + e + 640],
                                in_=ob[:])
            wo_stack.close()

    nc.compile()
    return nc


def _host_prep(x, Wq, Wk, Wv, Wo, q_norm_w, k_norm_w):
    bf = ml_dtypes.bfloat16
    x2 = np.asarray(x, np.float32).reshape(S, D_MODEL)
    xT = np.ascontiguousarray(x2.T).astype(bf)
    woT = np.ascontiguousarray(np.asarray(Wo, np.float32).T).astype(bf)

    # rope tables with norm weights + rotate-half sign folded in
    pos = np.arange(S, dtype=np.float64)
    inv_freq = 1.0 / (ROPE_THETA ** (np.arange(0, HEAD_DIM, 2, dtype=np.float64)
                                     / HEAD_DIM))
    ang = pos[:, None] * inv_freq[None, :]          # (S, 64)
    cos = np.concatenate([np.cos(ang), np.cos(ang)], axis=1)  # (S, 128)
    sin = np.concatenate([np.sin(ang), np.sin(ang)], axis=1)
    sgn = np.where(np.arange(HEAD_DIM) < 64, -1.0, 1.0)[None, :]
    rot_idx = (np.arange(HEAD_DIM) + 64) % HEAD_DIM
    qw = np.asarray(q_norm_w, np.float64)
    kw = np.asarray(k_norm_w, np.float64)
    cos_q = cos * qw[None, :]
    sin_q = sin * sgn * qw[rot_idx][None, :]
    cos_k = cos * kw[None, :]
    sin_k = sin * sgn * kw[rot_idx][None, :]
    cosT = np.concatenate([np.tile(cos_q, (1, Q_PER_CORE)), cos_k], axis=1).astype(bf)
    sinT = np.concatenate([np.tile(sin_q, (1, Q_PER_CORE)), sin_k], axis=1).astype(bf)

    tri = (np.triu(np.ones((128, 128), np.float32), k=1) * -60.0)  # -60 where sk > sq

    Wq = np.asarray(Wq, np.float32)
    Wk = np.asarray(Wk, np.float32)
    Wv = np.asarray(Wv, np.float32)
    in_maps = []
    for i in range(N_CORES):
        w_i = np.concatenate([
            Wq[Q_PER_CORE * HEAD_DIM * i: Q_PER_CORE * HEAD_DIM * (i + 1)],
            Wk[HEAD_DIM * i: HEAD_DIM * (i + 1)],
            Wv[HEAD_DIM * i: HEAD_DIM * (i + 1)],
        ], axis=0)                                   # (768, 2560)
        wqkvT = np.ascontiguousarray(w_i.T).astype(bf)
        in_maps.append({
            "xT": xT, "wqkvT": wqkvT, "woT": woT,
            "cosT": cosT, "sinT": sinT, "tri": tri,
        })
    return in_maps


def kernel(x, Wq, Wk, Wv, Wo, q_norm_w, k_norm_w, _trace=False, _trace_out=None):
    if "nc" not in _CACHE:
        _CACHE["nc"] = _build()
    nc = _CACHE["nc"]
    in_maps = _host_prep(x, Wq, Wk, Wv, Wo, q_norm_w, k_norm_w)
    kw = {}
    if _trace:
        kw = dict(trace=True)
        if _trace_out:
            kw["tmpdir"] = _trace_out
    res = run_bass_kernel_spmd(nc, in_maps, list(range(N_CORES)), **kw)
    _CACHE["last_exec_ns"] = res.exec_time_ns
    r = res.results
    out = np.concatenate([r[i]["out_slice"] for i in range(N_CORES)], axis=0)
    out = out.reshape(1, S, D_MODEL)
    kc = np.stack([r[i]["k_cache"] for i in range(N_CORES)], axis=0)[None]
    vc = np.stack([r[i]["v_cache"] for i in range(N_CORES)], axis=0)[None]
    return (out, kc, vc)


# revision 11
# speedup vs baseline: 1.0294x; 1.0097x over previous
"""Distributed Trainium2 kernel for the attention block (8 NeuronCores).

Sharding: core i owns KV head i and Q heads [4i, 4i+4) (tensor parallel over
heads). Attention outputs are transposed on the fly and exchanged with an
AllToAll so each core computes the o_proj for its 256-row slice of the
sequence; host concatenates slices.

All matmuls run in bf16 with fp32 PSUM accumulation; softmax statistics and
norms in fp32. RoPE + QK RMS-norm are folded into precomputed tables and
per-row scales.
"""
from contextlib import ExitStack

import numpy as np
import ml_dtypes

import concourse.bass as bass
import concourse.bacc as bacc
import concourse.tile as tile
from concourse.tile import add_dep_helper
from concourse import mybir
from concourse.bass_utils import run_bass_kernel_spmd

BF16 = mybir.dt.bfloat16
F32 = mybir.dt.float32

N_CORES = 8
S = 2048
D_MODEL = 2560
HEAD_DIM = 128
NUM_HEADS = 32
NUM_KV_HEADS = 8
Q_PER_CORE = NUM_HEADS // N_CORES          # 4
QKV_R = Q_PER_CORE * HEAD_DIM + 2 * HEAD_DIM  # 768 rows per core (4q + k + v)
QK_R = Q_PER_CORE * HEAD_DIM + HEAD_DIM       # 640 (q + k, normed+roped)
ROPE_THETA = 5e6
EPS = 1e-6
N_ST = S // 128          # 16 seq tiles
N_DC = D_MODEL // 128    # 20 contraction chunks
S_SLICE = S // N_CORES   # 256 output rows per core
N_HD = (NUM_HEADS * HEAD_DIM) // 128  # 32 o_proj contraction chunks
AV_W = 132               # 128 v cols + 1 ones + 3 pad
E_HALF = D_MODEL // 2    # 1280

_CACHE = {}


def _build():
    nc = bacc.Bacc("TRN2", target_bir_lowering=False, debug=False,
                   num_devices=N_CORES)

    xT = nc.dram_tensor("xT", [D_MODEL, S], BF16, kind="ExternalInput")
    wqkvT = nc.dram_tensor("wqkvT", [D_MODEL, QKV_R], BF16, kind="ExternalInput")
    woT = nc.dram_tensor("woT", [NUM_HEADS * HEAD_DIM, D_MODEL], BF16,
                         kind="ExternalInput")
    cosT = nc.dram_tensor("cosT", [S, QK_R], BF16, kind="ExternalInput")
    sinT = nc.dram_tensor("sinT", [S, QK_R], BF16, kind="ExternalInput")
    tri = nc.dram_tensor("tri", [128, 128], F32, kind="ExternalInput")

    out_slice = nc.dram_tensor("out_slice", [S_SLICE, D_MODEL], F32,
                               kind="ExternalOutput")
    k_cache = nc.dram_tensor("k_cache", [S, HEAD_DIM], F32, kind="ExternalOutput")
    v_cache = nc.dram_tensor("v_cache", [S, HEAD_DIM], F32, kind="ExternalOutput")

    with tile.TileContext(nc) as tc:
        with (
            tc.tile_pool(name="persist", bufs=1) as pp,
            tc.tile_pool(name="vaug", bufs=N_ST) as vaug_pool,
            tc.tile_pool(name="dram", bufs=1, space="DRAM") as dram,
        ):
            # persistent tiles
            qkT_all = pp.tile([128, 5, S], BF16)    # [d, block(4q+k), s]
            v_sb = [vaug_pool.tile([128, HEAD_DIM], BF16, tag="vaug", name=f"vsb{t}")
                     for t in range(N_ST)]
            tri_sb = pp.tile([128, 128], F32)
            nc.sync.dma_start(out=tri_sb[:], in_=tri[:])
            eps_sb = pp.tile([128, 1], F32)
            nc.vector.memset(eps_sb, float(HEAD_DIM) * EPS)


            a2a_in = dram.tile([N_CORES, Q_PER_CORE * HEAD_DIM, S_SLICE], BF16)
            a2a_out = dram.tile([N_CORES, Q_PER_CORE * HEAD_DIM, S_SLICE], BF16)

            # ---------------- Phase A: QKV projection + norm + rope ----------
            with (
                tc.tile_pool(name="xt", bufs=N_DC) as xt_pool,
                tc.tile_pool(name="wk", bufs=N_DC) as wk_pool,
                tc.tile_pool(name="tabs", bufs=3) as tab_pool,
                tc.tile_pool(name="qkwork", bufs=4) as work_pool,
                tc.tile_pool(name="stats", bufs=6) as stat_pool,
                tc.tile_pool(name="apsum", bufs=3, space="PSUM") as apsum,
            ):
                xT_sb = [xt_pool.tile([128, S], BF16, tag="xt", name=f"xTs{d}")
                         for d in range(N_DC)]
                wq_sb = [wk_pool.tile([128, QKV_R], BF16, tag="wk", name=f"wqs{d}")
                         for d in range(N_DC)]
                for d in range(N_DC):
                    nc.sync.dma_start(out=xT_sb[d][:], in_=xT[128 * d:128 * (d + 1), :])
                    nc.sync.dma_start(out=wq_sb[d][:], in_=wqkvT[128 * d:128 * (d + 1), :])

                for t in range(N_ST):
                    ps = apsum.tile([128, QKV_R], F32, tag="qkvps")
                    for d in range(N_DC):
                        lhsT = xT_sb[d][:, 128 * t:128 * (t + 1)]
                        nc.tensor.matmul(ps[:, 0:512], lhsT, wq_sb[d][:, 0:512],
                                         start=(d == 0), stop=(d == N_DC - 1))
                        nc.tensor.matmul(ps[:, 512:QKV_R], lhsT, wq_sb[d][:, 512:QKV_R],
                                         start=(d == 0), stop=(d == N_DC - 1))

                    # v: fp32 cache out + bf16 for AV
                    v32 = work_pool.tile([128, HEAD_DIM], F32, tag="v32")
                    nc.vector.tensor_copy(v32[:], ps[:, 640:768])
                    nc.gpsimd.dma_start(out=v_cache[128 * t:128 * (t + 1), :],
                                        in_=v32[:])
                    nc.vector.tensor_copy(v_sb[t][:], v32[:])

                    # rms-norm stats for 4 q heads + k
                    sums = stat_pool.tile([128, 5], F32, tag="sums")
                    scr = work_pool.tile([128, 128], BF16, tag="sqscr")
                    for b in range(5):
                        nc.scalar.activation(
                            out=scr[:], in_=ps[:, 128 * b:128 * (b + 1)],
                            func=mybir.ActivationFunctionType.Square,
                            accum_out=sums[:, b:b + 1])
                    inv = stat_pool.tile([128, 5], F32, tag="inv")
                    nc.scalar.activation(out=inv[:], in_=sums[:],
                                         func=mybir.ActivationFunctionType.Sqrt,
                                         bias=eps_sb[:])
                    nc.vector.reciprocal(inv[:], inv[:])
                    # k scale needs * sqrt(HEAD_DIM) (q keeps 1/sqrt(d) fold)
                    nc.vector.tensor_scalar_mul(inv[:, 4:5], inv[:, 4:5],
                                                float(np.sqrt(HEAD_DIM)))

                    # evacuate q,k with norm scale applied -> bf16
                    qk_n = work_pool.tile([128, QK_R], BF16, tag="qkn")
                    for b in range(5):
                        nc.scalar.activation(
                            out=qk_n[:, 128 * b:128 * (b + 1)],
                            in_=ps[:, 128 * b:128 * (b + 1)],
                            func=mybir.ActivationFunctionType.Copy,
                            scale=inv[:, b:b + 1])

                    # rope
                    cos_sb = tab_pool.tile([128, QK_R], BF16, tag="cos")
                    sin_sb = tab_pool.tile([128, QK_R], BF16, tag="sin")
                    nc.sync.dma_start(out=cos_sb[:], in_=cosT[128 * t:128 * (t + 1), :])
                    nc.sync.dma_start(out=sin_sb[:], in_=sinT[128 * t:128 * (t + 1), :])
                    qk_r = work_pool.tile([128, QK_R], BF16, tag="qkr")
                    tmp = work_pool.tile([128, QK_R], BF16, tag="ropetmp")
                    qn3 = qk_n.rearrange("p (b h) -> p b h", b=5)
                    qr3 = qk_r.rearrange("p (b h) -> p b h", b=5)
                    sn3 = sin_sb.rearrange("p (b h) -> p b h", b=5)
                    nc.vector.tensor_mul(qr3[:, :, 0:64], qn3[:, :, 64:128],
                                         sn3[:, :, 0:64])
                    nc.vector.tensor_mul(qr3[:, :, 64:128], qn3[:, :, 0:64],
                                         sn3[:, :, 64:128])
                    nc.vector.tensor_mul(tmp[:], qk_n[:], cos_sb[:])
                    nc.vector.tensor_add(qk_r[:], qk_r[:], tmp[:])

                    # k cache (fp32)
                    k32 = work_pool.tile([128, HEAD_DIM], F32, tag="k32")
                    nc.vector.tensor_copy(k32[:], qk_r[:, 512:640])
                    nc.gpsimd.dma_start(out=k_cache[128 * t:128 * (t + 1), :],
                                        in_=k32[:])

                    # batched transpose: all 5 blocks in one xbar call
                    nc.sync.dma_start_transpose(
                        out=qkT_all[:, :, 128 * t:128 * (t + 1)], in_=qk_r[:])

            # ---------------- Phase B: attention (natural softmax, v-stationary AV)
            wo_stack = ExitStack()
            wo_pool = wo_stack.enter_context(tc.tile_pool(name="wo", bufs=20))
            with (
                tc.tile_pool(name="ptt", bufs=1) as ptt_pool,
                tc.tile_pool(name="pt", bufs=4) as pt_pool,
                tc.tile_pool(name="asb", bufs=2) as a_pool,
                tc.tile_pool(name="den", bufs=8) as den_pool,
                tc.tile_pool(name="spsum", bufs=2, space="PSUM") as spsum,
                tc.tile_pool(name="avpsum", bufs=1, space="PSUM") as avpsum,
            ):
                # P^T storage: c-slot layout, slot c holds blocks (c, t>=c) at
                # col c*S + (t-c)*128  -> strip-t transpose writes with uniform
                # j-stride of (S - 128) elements; AV reads slot c contiguously.
                ptt = ptt_pool.tile([128, N_ST * S], BF16, name="ptt")
                ptt3 = ptt.rearrange("p (c w) -> p c w", c=N_ST)
                prev_av_last = None
                for h in range(Q_PER_CORE):
                    tr_by_t = []
                    for t in range(N_ST):
                        width = 128 * (t + 1)
                        pt = pt_pool.tile([128, 2048], BF16, tag="pt",
                                          name=f"pt{h}_{t}")
                        denp = den_pool.tile([128, 2], F32, tag="denp",
                                             name=f"denp{h}_{t}")
                        nch = (width + 1023) // 1024
                        for jc in range(nch):
                            j0 = 1024 * jc
                            w = min(1024, width - j0)
                            sps = spsum.tile([128, 1024], F32, tag="sps")
                            for e in range(0, w, 512):
                                we = min(512, w - e)
                                nc.tensor.matmul(
                                    sps[:, e:e + we],
                                    qkT_all[:, h, 128 * t:128 * (t + 1)],
                                    qkT_all[:, 4, j0 + e:j0 + e + we],
                                    start=True, stop=True)
                            if jc == nch - 1:
                                # additive causal mask on the diagonal block
                                nc.vector.tensor_add(sps[:, w - 128:w],
                                                     sps[:, w - 128:w], tri_sb[:])
                            nc.scalar.activation(
                                out=pt[:, j0:j0 + w], in_=sps[:, 0:w],
                                func=mybir.ActivationFunctionType.Exp,
                                accum_out=denp[:, jc:jc + 1])
                        den = den_pool.tile([128, 1], F32, tag="den",
                                            name=f"den{h}_{t}")
                        if nch == 2:
                            nc.vector.tensor_add(den[:], denp[:, 0:1], denp[:, 1:2])
                            nc.vector.reciprocal(den[:], den[:])
                        else:
                            nc.vector.reciprocal(den[:], denp[:, 0:1])
                        nc.vector.tensor_scalar_mul(pt[:, 0:width], pt[:, 0:width],
                                                    den[:])
                        # one xbar call: strip t -> P^T blocks (c=0..t, t)
                        eng = nc.sync
                        base = ptt[:]
                        dst = bass.AP(
                            tensor=base.tensor, offset=base.offset + 128 * t,
                            ap=[[base.ap[0][0], 128], [S - 128, t + 1], [1, 128]])
                        tr = eng.dma_start_transpose(out=dst, in_=pt[:, 0:width])
                        if prev_av_last is not None:
                            for c in range(t + 1):
                                add_dep_helper(tr.ins, prev_av_last[c].ins,
                                               reason="ptt WAR across heads")
                        tr_by_t.append(tr)

                    # AV: v_c stationary, wide P^T streams -> out^T accumulate
                    avT = avpsum.tile([128, S], F32, tag="avt", name=f"avt{h}")
                    av_last = [None] * N_ST
                    for c in range(N_ST):
                        sq = 128 * c
                        while sq < S:
                            w = min(512 - (sq % 512), S - sq)
                            last_c = min(N_ST - 1, 4 * (sq // 512) + 3)
                            mm = nc.tensor.matmul(
                                avT[:, sq:sq + w], v_sb[c][:],
                                ptt3[:, c, sq - 128 * c:sq - 128 * c + w],
                                start=(c == 0), stop=(c == last_c))
                            for t2 in range(sq // 128, (sq + w - 1) // 128 + 1):
                                add_dep_helper(mm.ins, tr_by_t[t2].ins,
                                               reason="AV reads ptt transposes")
                            av_last[c] = mm
                            sq += w
                    prev_av_last = av_last
                    at_h = a_pool.tile([128, S], BF16, tag="ath", name=f"ath{h}")
                    nc.vector.tensor_copy(at_h[:], avT[:])
                    for j in range(N_CORES):
                        nc.gpsimd.dma_start(
                            out=a2a_in[j, 128 * h:128 * (h + 1), :],
                            in_=at_h[:, S_SLICE * j:S_SLICE * (j + 1)])

                nc.gpsimd.collective_compute(
                    "AllToAll", mybir.AluOpType.bypass,
                    replica_groups=[list(range(N_CORES))],
                    ins=[a2a_in.opt()], outs=[a2a_out.opt()])

            # ---------------- Phase C: o_proj for our seq slice -------------
            with (
                tc.tile_pool(name="at", bufs=N_HD) as at_pool,
                tc.tile_pool(name="osb", bufs=4) as o_pool,
                tc.tile_pool(name="opsum", bufs=2, space="PSUM") as opsum,
            ):
                at_tiles = []
                for m in range(N_HD):
                    j, mm = divmod(m, Q_PER_CORE)
                    at = at_pool.tile([128, S_SLICE], BF16, tag="at",
                                      name=f"at{m}")
                    at_tiles.append(at)
                    nc.gpsimd.dma_start(
                        out=at[:], in_=a2a_out[j, 128 * mm:128 * (mm + 1), :])

                for eh in range(2):
                    e0 = eh * E_HALF
                    psos = [opsum.tile([128, E_HALF], F32, tag="pso",
                                       name=f"pso{eh}_{sc}") for sc in range(2)]
                    for m in range(N_HD):
                        wo_sb = wo_pool.tile([128, E_HALF], BF16, tag="wo",
                                             name=f"wo{eh}_{m}")
                        weng = nc.sync if m % 2 == 0 else nc.gpsimd
                        weng.dma_start(out=wo_sb[:],
                                       in_=woT[128 * m:128 * (m + 1),
                                               e0:e0 + E_HALF])
                        for sc in range(2):
                            for e in range(0, E_HALF, 512):
                                w = min(512, E_HALF - e)
                                nc.tensor.matmul(
                                    psos[sc][:, e:e + w],
                                    at_tiles[m][:, 128 * sc:128 * (sc + 1)],
                                    wo_sb[:, e:e + w],
                                    start=(m == 0), stop=(m == N_HD - 1))
                    for sc in range(2):
                        for e in range(0, E_HALF, 640):
                            ob = o_pool.tile([128, 640], F32, tag="osb")
                            nc.vector.tensor_copy(ob[:], psos[sc][:, e:e + 640])
                            nc.sync.dma_start(
                                out=out_slice[128 * sc:128 * (sc + 1),
                                              e0 + e:e0 + e + 640],
                                in_=ob[:])
            wo_stack.close()

    nc.compile()
    return nc


def _host_prep(x, Wq, Wk, Wv, Wo, q_norm_w, k_norm_w):
    bf = ml_dtypes.bfloat16
    x2 = np.asarray(x, np.float32).reshape(S, D_MODEL)
    xT = np.ascontiguousarray(x2.T).astype(bf)
    woT = np.ascontiguousarray(np.asarray(Wo, np.float32).T).astype(bf)

    # rope tables with norm weights + rotate-half sign folded in
    pos = np.arange(S, dtype=np.float64)
    inv_freq = 1.0 / (ROPE_THETA ** (np.arange(0, HEAD_DIM, 2, dtype=np.float64)
                                     / HEAD_DIM))
    ang = pos[:, None] * inv_freq[None, :]          # (S, 64)
    cos = np.concatenate([np.cos(ang), np.cos(ang)], axis=1)  # (S, 128)
    sin = np.concatenate([np.sin(ang), np.sin(ang)], axis=1)
    sgn = np.where(np.arange(HEAD_DIM) < 64, -1.0, 1.0)[None, :]
    rot_idx = (np.arange(HEAD_DIM) + 64) % HEAD_DIM
    qw = np.asarray(q_norm_w, np.float64)
    kw = np.asarray(k_norm_w, np.float64)
    cos_q = cos * qw[None, :]
    sin_q = sin * sgn * qw[rot_idx][None, :]
    cos_k = cos * kw[None, :]
    sin_k = sin * sgn * kw[rot_idx][None, :]
    cosT = np.concatenate([np.tile(cos_q, (1, Q_PER_CORE)), cos_k], axis=1).astype(bf)
    sinT = np.concatenate([np.tile(sin_q, (1, Q_PER_CORE)), sin_k], axis=1).astype(bf)

    tri = (np.triu(np.ones((128, 128), np.float32), k=1) * -60.0)  # -60 where sk > sq

    Wq = np.asarray(Wq, np.float32)
    Wk = np.asarray(Wk, np.float32)
    Wv = np.asarray(Wv, np.float32)
    in_maps = []
    for i in range(N_CORES):
        w_i = np.concatenate([
            Wq[Q_PER_CORE * HEAD_DIM * i: Q_PER_CORE * HEAD_DIM * (i + 1)],
            Wk[HEAD_DIM * i: HEAD_DIM * (i + 1)],
            Wv[HEAD_DIM * i: HEAD_DIM * (i + 1)],
        ], axis=0)                                   # (768, 2560)
        wqkvT = np.ascontiguousarray(w_i.T).astype(bf)
        in_maps.append({
            "xT": xT, "wqkvT": wqkvT, "woT": woT,
            "cosT": cosT, "sinT": sinT, "tri": tri,
        })
    return in_maps


def kernel(x, Wq, Wk, Wv, Wo, q_norm_w, k_norm_w, _trace=False, _trace_out=None):
    if "nc" not in _CACHE:
        _CACHE["nc"] = _build()
    nc = _CACHE["nc"]
    in_maps = _host_prep(x, Wq, Wk, Wv, Wo, q_norm_w, k_norm_w)
    kw = {}
    if _trace:
        kw = dict(trace=True)
        if _trace_out:
            kw["tmpdir"] = _trace_out
    res = run_bass_kernel_spmd(nc, in_maps, list(range(N_CORES)), **kw)
    _CACHE["last_exec_ns"] = res.exec_time_ns
    r = res.results
    out = np.concatenate([r[i]["out_slice"] for i in range(N_CORES)], axis=0)
    out = out.reshape(1, S, D_MODEL)
    kc = np.stack([r[i]["k_cache"] for i in range(N_CORES)], axis=0)[None]
    vc = np.stack([r[i]["v_cache"] for i in range(N_CORES)], axis=0)[None]
    return (out, kc, vc)
